# revision 10
# baseline (speedup 1.0000x reference)
"""MHSA Trainium2 kernel: B=2, N=2048, H=1024, 16 heads x d=64, fp32.

Sharding: 8 cores = 2 (batch) x 4 (head-groups of 4 heads). Each core is
fully independent (no collectives); host gathers per-core [256, 2048]
transposed outputs into [2, 2048, 1024].

Per-core device plan (all layouts chosen so softmax runs in the
"scores-transposed" orientation: j (keys) on partitions, i (queries) free):
  - inputs: hsT [1024,2048] (host-pretransposed), wqk [1024,512]
    (cols = q0|q1|q2|q3|k0|k1|k2|k3 per-head 64), wv [1024,256], biasj [2048]
    (0 or -30000 additive mask bias).
  - QK projection -> QT/KT per head in [d, tok] layout, duplicated into both
    partition halves so score matmuls can row-tile two j-tiles concurrently
    (contraction d=64 only fills half the PE rows).
  - V projection -> V_aug tiles [tok=128, 4*65] with a ones column per head:
    the attention matmul out = V_aug^T @ P^T (M=65) accumulates the softmax
    denominator in output row 64 for free.
  - scores^T = KT^T @ QT per (head, j-tile), exp via ACT with fused
    scale+mask-bias (per-partition bias = per-key mask), P^T -> SBUF.
  - normalize: reciprocal of l, broadcast across 64 partitions via a K=1
    matmul with a ones vector, multiply, DMA out.
"""

import numpy as np

import concourse.bass as bass
import concourse.bacc as bacc
import concourse.mybir as mybir
import concourse.tile as tile
from concourse.bass_utils import run_bass_kernel_spmd

F32 = mybir.dt.float32
F32R = mybir.dt.float32r
AF = mybir.ActivationFunctionType

HID = 1024
NT = 2048
D = 64
HPC = 4  # heads per core
NCORES = 8
SCALE = float(HID) ** -0.5
KD = HID // 128  # 8 contraction chunks
NJT = NT // 128  # 16 j-tiles
IB = 1024  # i-block
NIB = NT // IB

_CACHE = {}


def _build():
    if "nc" in _CACHE:
        return _CACHE["nc"]
    nc = bacc.Bacc("TRN2", debug=False)
    hsT_d = nc.dram_tensor("hsT", [HID, NT], F32R, kind="ExternalInput")
    wqk_d = nc.dram_tensor("wqk", [HID, 8 * D], F32R, kind="ExternalInput")
    wv_d = nc.dram_tensor("wv", [HID, HPC * D], F32R, kind="ExternalInput")
    bias_d = nc.dram_tensor("biasj", [NT], F32, kind="ExternalInput")
    outT_d = nc.dram_tensor("outT", [HPC * D, NT], F32, kind="ExternalOutput")

    with tile.TileContext(nc) as tc:
        with tc.tile_pool(name="per", bufs=1) as per:
            QTd = [per.tile([128, NT], F32R, tag=f"qtd{h}", name=f"qtd{h}") for h in range(HPC)]
            KTd = [per.tile([128, NT], F32R, tag=f"ktd{h}", name=f"ktd{h}") for h in range(HPC)]
            Vau = [per.tile([128, HPC, 65], F32R, tag=f"vau{t}", name=f"vau{t}") for t in range(NJT)]
            bias_t = per.tile([128, NJT], F32, tag="bias")
            ones64 = per.tile([1, D], F32R, tag="ones")
            nc.vector.memset(ones64[:].bitcast(F32), 1.0)
            nc.sync.dma_start(
                out=bias_t[:], in_=bias_d.ap().rearrange("(a p) -> p a", p=128)
            )
            for t in range(NJT):
                nc.vector.memset(Vau[t][:].bitcast(F32), 1.0)

            with (
                tc.tile_pool(name="ld", bufs=1) as ld,
                tc.tile_pool(name="pp", bufs=1, space="PSUM") as pp,
                tc.tile_pool(name="ppv", bufs=2, space="PSUM") as ppv,
            ):
                hsT = [ld.tile([128, NT], F32R, tag=f"hst{k}", name=f"hst{k}") for k in range(KD)]
                wqk = [ld.tile([128, 8 * D], F32R, tag=f"wqk{k}", name=f"wqk{k}") for k in range(KD)]
                wv = [ld.tile([128, HPC * D], F32R, tag=f"wv{k}", name=f"wv{k}") for k in range(KD)]
                hsT_r = hsT_d.ap().rearrange("(n p) m -> n p m", p=128)
                wqk_r = wqk_d.ap().rearrange("(n p) m -> n p m", p=128)
                wv_r = wv_d.ap().rearrange("(n p) m -> n p m", p=128)
                for k in range(KD):
                    nc.sync.dma_start(out=wqk[k][:], in_=wqk_r[k])
                    nc.sync.dma_start(out=wv[k][:], in_=wv_r[k])
                    nc.sync.dma_start(out=hsT[k][:], in_=hsT_r[k])

                # QK projection. chunk c: 0=[q0|q1] 1=[q2|q3] 2=[k0|k1] 3=[k2|k3]
                for c in range(4):
                    acc = [pp.tile([128, 512], F32, tag=f"pqk{t}", name=f"pqk{c}_{t}") for t in range(4)]
                    for k in range(KD):
                        for t in range(4):
                            nc.tensor.matmul(
                                acc[t][:],
                                wqk[k][:, c * 128 : (c + 1) * 128],
                                hsT[k][:, t * 512 : (t + 1) * 512],
                                start=(k == 0),
                                stop=(k == KD - 1),
                            )
                    dst = QTd if c < 2 else KTd
                    h0 = (c % 2) * 2
                    for t in range(4):
                        nc.vector.tensor_copy(
                            dst[h0][0:64, t * 512 : (t + 1) * 512],
                            acc[t][0:64, :],
                        )
                        nc.vector.tensor_copy(
                            dst[h0 + 1][64:128, t * 512 : (t + 1) * 512],
                            acc[t][64:128, :],
                        )
                # duplicate the filled half into the other partition half
                for h in range(HPC):
                    for dst in (QTd, KTd):
                        if h % 2 == 0:
                            nc.sync.dma_start(
                                out=dst[h][64:128, :], in_=dst[h][0:64, :]
                            )
                        else:
                            nc.sync.dma_start(
                                out=dst[h][0:64, :], in_=dst[h][64:128, :]
                            )

                # V projection: V_aug[t][:, h, 0:64] = v_h rows, col 64 stays 1.0
                for t in range(NJT):
                    pv = ppv.tile([128, HPC * D], F32, tag="pv")
                    for k in range(KD):
                        nc.tensor.matmul(
                            pv[:],
                            hsT[k][:, t * 128 : (t + 1) * 128],
                            wv[k][:],
                            start=(k == 0),
                            stop=(k == KD - 1),
                        )
                    for hh in range(HPC):
                        nc.vector.tensor_copy(
                            Vau[t][:, hh, 0:64], pv[:, hh * D : (hh + 1) * D]
                        )

            # attention
            with (
                tc.tile_pool(name="psc", bufs=3, space="PSUM") as psc,
                tc.tile_pool(name="psv", bufs=1, space="PSUM") as psv,
                tc.tile_pool(name="ptp", bufs=4) as ptp,
                tc.tile_pool(name="stg", bufs=2) as stg,
            ):
                for h in range(HPC):
                    for ib in range(NIB):
                        i0 = ib * IB
                        vout = psv.tile([128, IB], F32, tag="vout")
                        for jtp in range(NJT // 2):
                            jt0, jt1 = 2 * jtp, 2 * jtp + 1
                            sA = psc.tile([128, IB], F32, tag="sc")
                            sB = psc.tile([128, IB], F32, tag="sc")
                            for ic in range(IB // 512):
                                cs = slice(ic * 512, (ic + 1) * 512)
                                qs = slice(i0 + ic * 512, i0 + (ic + 1) * 512)
                                nc.tensor.matmul(
                                    sA[:, cs],
                                    KTd[h][0:64, jt0 * 128 : (jt0 + 1) * 128],
                                    QTd[h][0:64, qs],
                                    start=True,
                                    stop=True,
                                    tile_position=(0, 0),
                                )
                                nc.tensor.matmul(
                                    sB[:, cs],
                                    KTd[h][64:128, jt1 * 128 : (jt1 + 1) * 128],
                                    QTd[h][64:128, qs],
                                    start=True,
                                    stop=True,
                                    tile_position=(64, 0),
                                )
                            ptA = ptp.tile([128, IB], F32R, tag="pt")
                            ptB = ptp.tile([128, IB], F32R, tag="pt")
                            nc.scalar.activation(
                                ptA[:], sA[:], AF.Exp,
                                bias=bias_t[:, jt0 : jt0 + 1], scale=SCALE,
                            )
                            nc.scalar.activation(
                                ptB[:], sB[:], AF.Exp,
                                bias=bias_t[:, jt1 : jt1 + 1], scale=SCALE,
                            )
                            for jt, pt in ((jt0, ptA), (jt1, ptB)):
                                for ic in range(IB // 512):
                                    cs = slice(ic * 512, (ic + 1) * 512)
                                    nc.tensor.matmul(
                                        vout[0:65, cs],
                                        Vau[jt][:, h, :],
                                        pt[:, cs],
                                        start=(jt == 0),
                                        stop=(jt == NJT - 1),
                                    )
                        # normalize: row 64 of vout is l(i)
                        vo = stg.tile([65, IB], F32, tag="vo")
                        nc.vector.tensor_copy(vo[:], vout[0:65, :])
                        rl = stg.tile([1, IB], F32R, tag="rl")
                        with nc.allow_low_precision("f32r is bit-identical to f32"):
                            nc.vector.reciprocal(rl[:], vo[64:65, :])
                        rlb = psc.tile([64, IB], F32, tag="sc")
                        for ic in range(IB // 512):
                            cs = slice(ic * 512, (ic + 1) * 512)
                            nc.tensor.matmul(
                                rlb[:, cs], ones64[:], rl[:, cs],
                                start=True, stop=True,
                            )
                        ot = stg.tile([64, IB], F32, tag="ot")
                        nc.vector.tensor_mul(ot[:], vo[0:64, :], rlb[:])
                        nc.sync.dma_start(
                            out=outT_d.ap()[h * D : (h + 1) * D, i0 : i0 + IB],
                            in_=ot[:],
                        )
    if not nc.is_finalized():
        nc.finalize()
    _CACHE["nc"] = nc
    return nc


def kernel(hidden_states, attention_mask, W_qkv):
    hs = np.asarray(hidden_states, dtype=np.float32)  # [2, 2048, 1024]
    am = np.asarray(attention_mask)  # [2, 2048]
    W = np.asarray(W_qkv, dtype=np.float32)  # [16, 1024, 192]

    nc = _build()
    in_maps = []
    for core in range(NCORES):
        b, hg = core // 4, core % 4
        Wc = W[hg * 4 : hg * 4 + 4]  # [4, 1024, 192]
        q = [Wc[h, :, 0:64] for h in range(4)]
        k = [Wc[h, :, 64:128] for h in range(4)]
        v = [Wc[h, :, 128:192] for h in range(4)]
        in_maps.append(
            {
                "hsT": np.ascontiguousarray(hs[b].T),
                "wqk": np.ascontiguousarray(np.concatenate(q + k, axis=1)),
                "wv": np.ascontiguousarray(np.concatenate(v, axis=1)),
                "biasj": ((am[b] != 0).astype(np.float32) - 1.0) * 30000.0,
            }
        )
    res = run_bass_kernel_spmd(nc, in_maps, list(range(NCORES)))
    if res.exec_time_ns is not None:
        print(f"HW exec time: {res.exec_time_ns} ns")
    if res.mean_exec_time_ns is not None:
        print(f"HW exec time (mean across cores): {res.mean_exec_time_ns} ns")
    out = np.empty((2, NT, HID), dtype=np.float32)
    for core in range(NCORES):
        b, hg = core // 4, core % 4
        out[b, :, hg * 256 : (hg + 1) * 256] = res.results[core]["outT"].T
    return out


def predicted_exec_ns():
    """Device-occupancy estimate for one core (all 8 run the same program
    in parallel). Used by test.py; the real NTFF profiling hook is not
    available in this container."""
    nc = _build()
    from concourse.timeline_sim import TimelineSim
    return float(TimelineSim(nc, trace=False).simulate())
